# revision 19
# baseline (speedup 1.0000x reference)
"""CondConv kernel for 8 Trainium2 NeuronCores.

Problem: per-sample conditional conv.
  att[b] = sigmoid(MLP(concat(z[b], avgpool(x[b]))))          # [B, K=4]
  agg_w[b] = sum_k att[b,k] * 0.1 * weight[k] + static_weight # [O, C, 3, 3]
  out[b] = conv2d(x[b], agg_w[b], pad=1)                      # [O, H, W]

Sharding: data-parallel over batch. B=32 -> 4 samples per core; the
K-kernel weight bank and attention params are replicated.

Device strategy per core (4 local samples):
  - weights pre-packed on host to [c, (kh,kw,o)] bf16 layout (lhsT-ready)
  - DMA transfers are serial at ~358GB/s, so ordering is the prologue:
    x0 raced on the ACT+SP queues, then the cc0 weight-bank halves in
    mixing-consumption order, x1, the cc1 bank, x2, x3
  - attention entirely in bf16 (4x faster PE matmuls than fp32); the
    MLP params ride one merged bf16 tensor
  - per-sample mixed weights agg = s0*w0+s1*w1+s2*w2+s3*w3+static:
    ACT m1=s1*w1, m2=s2*w2; Pool b3 = w3*s3 + m2 (fused STT);
    DVE agg = w0*s0 + static (STT), agg += m1, agg += b3 (2x bf16 adds)
  - conv as implicit GEMM: per (o_chunk, hw_half) 18 accumulating
    matmuls (2 c-chunks x 9 taps) of [128c x 128o x 512hw] bf16, fp32
    PSUM.  Sample 0 runs the cc0 taps of all four PSUM groups first,
    then the cc1 taps trickle in behind the cc1 bank DMA.
  - emission is software-pipelined around the in-order engine queues
  - conv output staged in bf16 (halves the out-DMA bytes; host converts)
  - pad buffers get border-only memsets (interior is fully overwritten)
"""

import sys

if "/opt/trn_rl_repo" not in sys.path:
    sys.path.insert(0, "/opt/trn_rl_repo")

import ml_dtypes
import numpy as np

B, C, H, W = 32, 256, 32, 32
T, O, K, KS = 256, 256, 4, 3
EPS = 1e-5
NCORES = 8
BL = B // NCORES          # 4 local samples per core
HWF = H * W               # 1024
JN = KS * KS              # 9 taps
FW = JN * O               # 2304 free width of packed weight tiles
PC = 128                  # partitions
CC = C // PC              # 2 c chunks
OC = O // PC              # 2 o chunks
NHALF = HWF // 512        # 2 hw halves of 512
NZW = K + BL              # bf16 params columns appended to w1t: w2t | zt
WA = T + NZW              # merged w1all column count
FP8_BANK = True           # K-kernel bank in fp8e4m3 (x64 host prescale);
                          # static stays bf16, 1/64 folded into att scales
FP8_SCALE = 64.0

_CACHE = {}


def _build_module(reps=1):
    import concourse.mybir as mybir
    from concourse import bacc
    from concourse.tile import TileContext

    nc = bacc.Bacc("TRN2", target_bir_lowering=False)

    x_d = nc.dram_tensor("x", [BL, C, H, W], mybir.dt.bfloat16, kind="ExternalInput")
    par_d = nc.dram_tensor("params", [T, 2], mybir.dt.float32, kind="ExternalInput")
    w1_d = nc.dram_tensor("w1all", [T + C, WA], mybir.dt.bfloat16,
                          kind="ExternalInput")
    wb_dt = mybir.dt.float8e4 if FP8_BANK else mybir.dt.bfloat16
    wb_d = nc.dram_tensor("wbank", [K, C, FW], wb_dt, kind="ExternalInput")
    ws_d = nc.dram_tensor("wstatic", [C, FW], mybir.dt.bfloat16, kind="ExternalInput")
    out_d = nc.dram_tensor(
        "out", [BL, O, HWF], mybir.dt.bfloat16, kind="ExternalOutput"
    )

    with TileContext(nc) as tc:
        if reps == 1:
            _emit_body(nc, tc, mybir, x_d, par_d, w1_d, wb_d, ws_d, out_d)
        else:
            engs = [
                mybir.EngineType.PE,
                mybir.EngineType.Activation,
                mybir.EngineType.DVE,
                mybir.EngineType.SP,
                mybir.EngineType.Pool,
            ]
            with tc.For_i(0, reps, 1, hint_engines=tuple(engs)):
                _emit_body(nc, tc, mybir, x_d, par_d, w1_d, wb_d, ws_d, out_d)

    nc.compile()
    return nc


def _emit_body(nc, tc, mybir, x_d, par_d, w1_d, wb_d, ws_d, out_d):
    dt = mybir.dt
    AF = mybir.ActivationFunctionType
    ALU = mybir.AluOpType
    with (
        tc.tile_pool(name="res", bufs=1) as res,
        tc.tile_pool(name="small", bufs=1) as small,
        tc.tile_pool(name="pads", bufs=4) as pads,
        tc.tile_pool(name="aggs", bufs=4) as aggs,
        tc.tile_pool(name="mix", bufs=2) as mix,
        tc.tile_pool(name="outs", bufs=2) as outs,
        tc.tile_pool(name="cps", bufs=6, space="PSUM") as cps,
        tc.tile_pool(name="sps", bufs=1, space="PSUM") as sps,
    ):
        w1_sb = res.tile([PC, 4, WA], dt.bfloat16, tag="w1all")
        par_sb = res.tile([PC, 2, 2], dt.float32, tag="par")
        sc_sb = [par_sb[:, t, 0:1] for t in range(2)]
        bi_sb = [par_sb[:, t, 1:2] for t in range(2)]
        w2t_sb = [w1_sb[:, t, T : T + K] for t in range(2)]
        zt_sb = [w1_sb[:, t, T + K : T + NZW] for t in range(2)]

        xst_all = [
            small.tile([PC, CC, H, W], dt.bfloat16, tag=f"xst{b}", name=f"xst{b}")
            for b in range(BL)
        ]
        ws_sb = [res.tile([PC, FW], dt.bfloat16, tag=f"wst{cc}", name=f"wst{cc}")
                 for cc in range(CC)]
        wdt = dt.float8e4 if FP8_BANK else dt.bfloat16
        w_sb = [
            [res.tile([PC, FW], wdt, tag=f"wk{k}c{cc}", name=f"wk{k}c{cc}")
             for cc in range(CC)]
            for k in range(K)
        ]

        # ---- DMA issues.  x0 halves race on the ACT/SP queues; then the
        # SP queue carries w1all/par, the cc0 bank halves in consumption
        # order (w1, w2, w0, static, w3), x1, the cc1 bank, x2, x3.
        # Transfers are serial, so this order IS the arrival schedule. ----
        nc.scalar.dma_start(
            out=xst_all[0][:, 0:1],
            in_=x_d[0, 0:PC].rearrange("(a p) h w -> p a h w", p=PC),
        )
        nc.sync.dma_start(
            out=xst_all[0][:, 1:2],
            in_=x_d[0, PC : 2 * PC].rearrange("(a p) h w -> p a h w", p=PC),
        )
        nc.sync.dma_start(out=w1_sb, in_=w1_d.rearrange("(a p) t -> p a t", p=PC))
        nc.sync.dma_start(out=par_sb, in_=par_d.rearrange("(t p) s -> p t s", p=PC))
        FH = FW // 2

        def bank_cc(cc):
            rows = slice(cc * PC, (cc + 1) * PC)
            for h in range(2):
                fs = slice(h * FH, (h + 1) * FH)
                for k in (1, 2, 0):
                    nc.sync.dma_start(out=w_sb[k][cc][:, fs],
                                      in_=wb_d[k, rows, fs])
                nc.sync.dma_start(out=ws_sb[cc][:, fs], in_=ws_d[rows, fs])
                nc.sync.dma_start(out=w_sb[3][cc][:, fs], in_=wb_d[3, rows, fs])

        def load_x(b):
            nc.sync.dma_start(
                out=xst_all[b], in_=x_d[b].rearrange("(a p) h w -> p a h w", p=PC)
            )

        load_x(1)
        bank_cc(0)
        bank_cc(1)
        load_x(2)
        load_x(3)

        # warm the Copy table set first so pad0's copy starts the moment
        # x0 lands; Relu/Sigmoid sets are warmed inside emit_pads_attn(0)
        # where their loads overlap the pad/avg window
        actwarm = res.tile([1, 3], dt.float32, tag="actwarm")
        nc.scalar.activation(out=actwarm[:, 0:1], in_=par_sb[0:1, 0, 0:1],
                             func=AF.Copy)

        pad_all = [[None] * CC for _ in range(BL)]
        s_all = [None] * BL

        def emit_pads_attn(b):
            """pad + avgpool + attention MLP for sample b -> s_all[b]."""
            avg_f = small.tile([PC, CC], dt.float32, tag=f"avgf{b}",
                               name=f"avgf{b}")
            for cc in range(CC):
                pad = pads.tile([PC, H + 2, W + 2], dt.bfloat16, tag=f"pad{cc}")
                # border-only memsets; interior fully overwritten below
                nc.gpsimd.memset(pad[:, 0, :], 0.0)
                nc.gpsimd.memset(pad[:, H + 1, :], 0.0)
                nc.gpsimd.memset(pad[:, 1 : 1 + H, 0:1], 0.0)
                nc.gpsimd.memset(pad[:, 1 : 1 + H, W + 1 : W + 2], 0.0)
                if cc == 1 and b == 0:
                    # sample 0's second pad-copy on DVE so the prologue's
                    # two copies don't serialize; the avg reduce goes
                    # first — it gates attention, the pad copy doesn't
                    nc.vector.tensor_reduce(
                        out=avg_f[:, cc : cc + 1],
                        in_=xst_all[b][:, cc],
                        axis=mybir.AxisListType.XY,
                        op=ALU.add,
                    )
                    nc.vector.tensor_copy(
                        pad[:, 1 : 1 + H, 1 : 1 + W], xst_all[b][:, cc]
                    )
                else:
                    nc.scalar.activation(
                        out=pad[:, 1 : 1 + H, 1 : 1 + W],
                        in_=xst_all[b][:, cc],
                        func=AF.Copy,
                        accum_out=avg_f[:, cc : cc + 1],
                    )
                pad_all[b][cc] = pad
            if b == 0:
                nc.scalar.activation(out=actwarm[:, 1:2],
                                     in_=par_sb[0:1, 0, 0:1], func=AF.Relu)
                nc.scalar.activation(out=actwarm[:, 2:3],
                                     in_=par_sb[0:1, 0, 0:1], func=AF.Sigmoid)

            avg_b = small.tile([PC, CC], dt.bfloat16, tag=f"avgb{b}",
                               name=f"avgb{b}")
            nc.vector.tensor_copy(avg_b, avg_f)

            a_srcs = [zt_sb[0], zt_sb[1], avg_b, avg_b]
            a_col = [slice(b, b + 1), slice(b, b + 1), slice(0, 1), slice(1, 2)]
            hT_ps = sps.tile([PC, 2], dt.float32, tag="hT")
            for t in range(2):
                for ic in range(4):
                    nc.tensor.matmul(
                        out=hT_ps[:, t : t + 1],
                        lhsT=w1_sb[:, ic, t * PC : (t + 1) * PC],
                        rhs=a_srcs[ic][:, a_col[ic]],
                        start=(ic == 0),
                        stop=(ic == 3),
                    )
            hrelu = small.tile([PC, 2], dt.bfloat16, tag=f"hrelu{b}",
                               name=f"hrelu{b}")
            for t in range(2):
                nc.scalar.activation(
                    out=hrelu[:, t : t + 1],
                    in_=hT_ps[:, t : t + 1],
                    func=AF.Relu,
                    scale=sc_sb[t],
                    bias=bi_sb[t],
                )
            att_ps = sps.tile([1, K], dt.float32, tag="attps")
            for t in range(2):
                nc.tensor.matmul(
                    out=att_ps,
                    lhsT=hrelu[:, t : t + 1],
                    rhs=w2t_sb[t],
                    start=(t == 0),
                    stop=(t == 1),
                )
            sig_sb = small.tile([1, K], dt.float32, tag=f"sig{b}", name=f"sig{b}")
            nc.scalar.activation(out=sig_sb, in_=att_ps, func=AF.Sigmoid)
            if FP8_BANK:
                # bank is host-prescaled by FP8_SCALE; fold 1/SCALE into s
                nc.vector.tensor_scalar_mul(
                    out=sig_sb, in0=sig_sb, scalar1=1.0 / FP8_SCALE
                )
            s_sb = small.tile([PC, K], dt.float32, tag=f"s{b}", name=f"s{b}")
            nc.gpsimd.partition_broadcast(s_sb, sig_sb)
            s_all[b] = s_sb

        agg_all = [None] * BL

        def emit_mix(b):
            """agg = s0*w0 + s1*w1 + s2*w2 + s3*w3 + static for sample b."""
            s_sb = s_all[b]
            agg_sb = []
            nsub = 4
            FHm = FW // nsub
            for cc in range(CC):
                m1 = mix.tile([PC, FW], dt.bfloat16, tag=f"m1c{cc}", name=f"m1c{cc}")
                m2 = mix.tile([PC, FW], dt.bfloat16, tag=f"m2c{cc}", name=f"m2c{cc}")
                b3 = mix.tile([PC, FW], dt.bfloat16, tag=f"b3c{cc}", name=f"b3c{cc}")
                agg = aggs.tile([PC, FW], dt.bfloat16, tag=f"agg{cc}",
                                name=f"agg{cc}")
                for h in range(nsub):
                    fs = slice(h * FHm, (h + 1) * FHm)
                    # Pool supports only plain tensor_tensor (no STT) and
                    # cannot touch PSUM, so it gets the m1+m2 pair-add
                    nc.scalar.activation(
                        out=m1[:, fs], in_=w_sb[1][cc][:, fs], func=AF.Copy,
                        scale=s_sb[:, 1:2],
                    )
                    nc.scalar.activation(
                        out=m2[:, fs], in_=w_sb[2][cc][:, fs], func=AF.Copy,
                        scale=s_sb[:, 2:3],
                    )
                    nc.gpsimd.tensor_tensor(
                        out=b3[:, fs], in0=m1[:, fs], in1=m2[:, fs], op=ALU.add
                    )
                    nc.vector.scalar_tensor_tensor(
                        out=agg[:, fs],
                        in0=w_sb[0][cc][:, fs],
                        scalar=s_sb[:, 0:1],
                        in1=ws_sb[cc][:, fs],
                        op0=ALU.mult,
                        op1=ALU.add,
                    )
                    nc.vector.scalar_tensor_tensor(
                        out=agg[:, fs],
                        in0=w_sb[3][cc][:, fs],
                        scalar=s_sb[:, 3:4],
                        in1=agg[:, fs],
                        op0=ALU.mult,
                        op1=ALU.add,
                    )
                    nc.vector.tensor_tensor(
                        out=agg[:, fs], in0=agg[:, fs], in1=b3[:, fs], op=ALU.add
                    )
                agg_sb.append(agg)
            agg_all[b] = agg_sb

        groups = [(oc, half) for oc in range(OC) for half in range(NHALF)]
        taps9 = [(kh, kw) for kh in range(KS) for kw in range(KS)]

        def emit_mm(b, ps, oc, half, cc, kh, kw, start, stop):
            j = kh * KS + kw
            nc.tensor.matmul(
                out=ps,
                lhsT=agg_all[b][cc][:, j * O + oc * PC : j * O + oc * PC + PC],
                rhs=pad_all[b][cc][
                    :, half * 16 + kh : half * 16 + kh + 16, kw : kw + W
                ],
                start=start,
                stop=stop,
            )

        def emit_copy(ps, osb, g, oc, half):
            # copy engines per group: ACT, DVE, DVE, ACT (Pool/GPSIMD
            # cannot read PSUM on real hardware)
            dst = osb[:, oc, half * 512 : (half + 1) * 512]
            if g in (1, 2):
                nc.vector.tensor_copy(dst, ps)
            else:
                nc.scalar.activation(out=dst, in_=ps, func=AF.Copy)

        def emit_conv_mm(b):
            """All 4 groups' matmuls for sample b; returns psum tiles."""
            pss = [
                cps.tile([PC, 512], dt.float32, tag="ps", name="ps")
                for _ in groups
            ]
            if b == 0:
                # cc0 taps of all four groups first, then cc1 taps trickle
                # in behind the cc1 bank DMA
                for ccl in range(CC):
                    for g, (oc, half) in enumerate(groups):
                        for j, (kh, kw) in enumerate(taps9):
                            emit_mm(b, pss[g], oc, half, ccl, kh, kw,
                                    start=(ccl == 0 and j == 0),
                                    stop=(ccl == CC - 1 and j == JN - 1))
            else:
                for g, (oc, half) in enumerate(groups):
                    n = 0
                    for cc in range(CC):
                        for kh, kw in taps9:
                            emit_mm(b, pss[g], oc, half, cc, kh, kw,
                                    start=(n == 0), stop=(n == 2 * JN - 1))
                            n += 1
            return pss

        def emit_copies_out(b, pss):
            osb = outs.tile([PC, OC, HWF], dt.bfloat16, tag="osb")
            out_r = out_d[b].rearrange("(a p) f -> p a f", p=PC)
            for g, (oc, half) in enumerate(groups):
                emit_copy(pss[g], osb, g, oc, half)
                if b == BL - 1:
                    nc.sync.dma_start(
                        out=out_r[:, oc, half * 512 : (half + 1) * 512],
                        in_=osb[:, oc, half * 512 : (half + 1) * 512],
                    )
                elif half == NHALF - 1:
                    nc.sync.dma_start(
                        out=out_r[:, oc : oc + 1, :], in_=osb[:, oc : oc + 1, :]
                    )

        # ---- software-pipelined schedule.  Samples 0-1's attention runs
        # up front (x0/x1 arrive first); samples 2-3's pads+attention are
        # enqueued between mixes, when their x has landed; each sample's
        # PSUM->SBUF copies are enqueued after a later sample's mixing so
        # the in-order vector queues never make mixing wait on conv ----
        emit_pads_attn(0)
        emit_pads_attn(1)
        emit_mix(0)
        emit_mix(1)
        pss0 = emit_conv_mm(0)
        emit_pads_attn(2)
        emit_pads_attn(3)
        emit_mix(2)
        emit_copies_out(0, pss0)
        pss1 = emit_conv_mm(1)
        emit_mix(3)
        emit_copies_out(1, pss1)
        pss2 = emit_conv_mm(2)
        emit_copies_out(2, pss2)
        pss3 = emit_conv_mm(3)
        emit_copies_out(3, pss3)


def _get_module(reps=1):
    key = ("nc", reps)
    if key not in _CACHE:
        _CACHE[key] = _build_module(reps)
    return _CACHE[key]


def _prep_shared(w1, bn_gamma, bn_beta, bn_mean, bn_var, w2, weight, static_weight):
    bf16 = ml_dtypes.bfloat16
    scale = (bn_gamma / np.sqrt(bn_var + EPS)).astype(np.float32)
    bias = (bn_beta - bn_mean * scale).astype(np.float32)
    w1t = np.ascontiguousarray(w1.T).astype(np.float32)  # [T+C, T]
    w1t[T:, :] *= np.float32(1.0 / HWF)  # device supplies raw sums, not means
    w2t = np.ascontiguousarray(w2.T).astype(np.float32)  # [T, K]
    wb = np.ascontiguousarray(
        (0.1 * weight).transpose(0, 2, 3, 4, 1).reshape(K, C, FW)
    )
    if FP8_BANK:
        wbank = (wb * FP8_SCALE).astype(ml_dtypes.float8_e4m3)
    else:
        wbank = wb.astype(bf16)
    wstatic = np.ascontiguousarray(
        static_weight.transpose(1, 2, 3, 0).reshape(C, FW)
    ).astype(bf16)
    par = np.zeros((T, 2), np.float32)
    par[:, 0] = scale
    par[:, 1] = bias
    w1all = np.zeros((T + C, WA), np.float32)
    w1all[:, :T] = w1t
    w1all[:T, T : T + K] = w2t
    return {"wbank": wbank, "wstatic": wstatic}, par, w1all


def make_in_maps(x, z, w1, bn_gamma, bn_beta, bn_mean, bn_var, w2, weight,
                 static_weight):
    bf16 = ml_dtypes.bfloat16
    shared, par, w1all = _prep_shared(
        w1, bn_gamma, bn_beta, bn_mean, bn_var, w2, weight, static_weight
    )
    in_maps = []
    for i in range(NCORES):
        lo = i * BL
        w1i = w1all.copy()
        w1i[:T, T + K : T + NZW] = z[lo : lo + BL].T
        in_maps.append(
            {
                "x": np.ascontiguousarray(x[lo : lo + BL]).astype(bf16),
                "params": par,
                "w1all": w1i.astype(bf16),
                **shared,
            }
        )
    return in_maps


def run(in_maps, reps=1, **kwargs):
    from concourse.bass_utils import run_bass_kernel_spmd

    nc = _get_module(reps)
    return run_bass_kernel_spmd(nc, in_maps, core_ids=list(range(NCORES)), **kwargs)


def kernel(**inputs):
    in_maps = make_in_maps(**inputs)
    res = run(in_maps)
    out = np.concatenate([r["out"].reshape(BL, O, H, W) for r in res.results], axis=0)
    return out.astype(np.float32)
